# revision 11
# baseline (speedup 1.0000x reference)
"""Cross multi-head attention Trainium2 Bass kernel.

Problem: nn_CrossMutiHeadAttention (B=4, SQ=SKV=2048, d_model=1024, H=8,
d_k=64, d_v=128), fp32 in/out.

Sharding (8 cores, no collectives): core c handles batch c//2 and query-row
half c%2 — each core computes K/V projections for its batch (duplicated
across the 2 cores sharing a batch) plus attention + output projection for
its 1024 query rows.

Per-core pipeline (everything SPMD-identical, host shards the inputs):
  ph1: transpose enc (PE) -> encT; project K^T (-> DRAM scratch, bf16) and
       V (-> SBUF, bf16) per 512-row block.
  ph2/3: transpose pre -> preT half; project Q^T (bf16, head-pair packed).
  ph4: per (q-half, head): S^T = K^T.T@Q^T into PSUM (f32), exp via ACT
       (scale=1/8) -> bf16, denominator via ones-matmul, O^T = V.T@P^T
       accumulated over kv chunks, normalize with broadcast reciprocal
       (-> f32r).
  ph5: Y = O^T.T @ Wo (f32r), ACT copy -> f32, DMA out.

The whole body sits in a For_i whose trip count is a runtime input so
test.py can measure steady-state device time via wall-clock deltas.
"""

from contextlib import ExitStack

import numpy as np

import concourse.bass as bass
import concourse.mybir as mybir
from concourse import bacc
from concourse.bass_utils import run_bass_kernel_spmd
from concourse.masks import make_identity
from concourse.tile import TileContext

F32 = mybir.dt.float32
F32R = mybir.dt.float32r
BF16 = mybir.dt.bfloat16

P = 128
B, SQ, SKV, DM = 4, 2048, 2048, 1024
H, DK, DV = 8, 64, 128
SQH = SQ // 2          # 1024 query rows per core
HP = H // 2            # 4 head pairs
CO = DM // P           # 8 contraction chunks
N_CORES = 8

EXP_SCALE = 1.0 / np.sqrt(DK).astype(np.float32)  # 0.125


def build(loop_phase="all"):
    nc = bacc.Bacc()
    enc = nc.declare_dram_parameter("enc", [SKV, DM], F32, isOutput=False)
    pre = nc.declare_dram_parameter("pre", [SQH, DM], F32, isOutput=False)
    wq = nc.declare_dram_parameter("wq", [DM, H * DK], F32, isOutput=False)
    wk = nc.declare_dram_parameter("wk", [DM, H * DK], F32, isOutput=False)
    wv = nc.declare_dram_parameter("wv", [DM, DM], F32, isOutput=False)
    wo = nc.declare_dram_parameter("wo", [DM, DM], F32, isOutput=False)
    n_it = nc.declare_dram_parameter("n_it", [1, 1], mybir.dt.uint32, isOutput=False)
    out = nc.declare_dram_parameter("out", [SQH, DM], F32, isOutput=True)

    with ExitStack() as ctx:
        tc = ctx.enter_context(TileContext(nc))
        ec = ctx.enter_context
        if True:
            cpool = ec(tc.tile_pool(name="const", bufs=1))
            stg_pool = ec(tc.tile_pool(name="stg", bufs=8))
            wk_pool = ec(tc.tile_pool(name="wk", bufs=1))
            wbig_pool = ec(tc.tile_pool(name="wbig", bufs=1))
            ktst_pool = ec(tc.tile_pool(name="ktst", bufs=2))
            ktw_pool = ec(tc.tile_pool(name="ktw", bufs=2))
            qt_pool = ec(tc.tile_pool(name="qt", bufs=1))
            tblk_pool = ec(tc.tile_pool(name="tblk", bufs=2))
            v_pool = ec(tc.tile_pool(name="vpool", bufs=1))
            exp_pool = ec(tc.tile_pool(name="exp", bufs=8))
            ot_pool = ec(tc.tile_pool(name="ot", bufs=1))
            y_pool = ec(tc.tile_pool(name="ysb", bufs=2))
            r_pool = ec(tc.tile_pool(name="rsm", bufs=2))
            dram_pool = ec(tc.tile_pool(name="dram", bufs=1, space="DRAM"))
            ps_st = ec(tc.tile_pool(name="ps_st", bufs=2, space="PSUM"))
            ps_mm = ec(tc.tile_pool(name="ps_mm", bufs=2, space="PSUM"))
            ps_aux = ec(tc.tile_pool(name="ps_aux", bufs=2, space="PSUM"))
            ident = cpool.tile([P, P], BF16, tag="ident")
            make_identity(nc, ident[:])
            ones = cpool.tile([P, 1], BF16, tag="ones")
            nc.gpsimd.memset(ones[:], 1.0)
            nit_sb = cpool.tile([1, 1], mybir.dt.uint32, tag="nit")
            nc.sync.dma_start(nit_sb[:], n_it[:])

            regs = []
            for eng_t in mybir.ALL_ENGINES:
                r = nc.alloc_register(eng_t, f"nit_{eng_t.name}")
                nc.engines[eng_t].reg_load(r, nit_sb[0:1, 0:1])
                regs.append(r)
            n_val = bass.RegisterHandles(regs)

            state = {}

            def ph1():
                # K^T scratch: [pair, 128 (d of 2 heads), SKV] bf16
                kt_dram = dram_pool.tile([HP, P, SKV], BF16, tag="kt")
                state["kt_dram"] = kt_dram
                # first enc block DMA goes out ahead of the weight loads so
                # PE transposes can start immediately
                stg0 = []
                for t in range(4):
                    sg = stg_pool.tile([P, 1, DM], F32, tag="stg")
                    nc.sync.dma_start(
                        sg[:, 0, :],
                        enc[t * P : (t + 1) * P],
                    )
                    stg0.append(sg)
                # ---- weights for ph1 (f32 DMA + DVE cast to bf16) ----
                wk_sb = wk_pool.tile([P, CO, H * DK], BF16, tag="wk")
                for co4 in range(4):
                    wstg = stg_pool.tile([P, 2, H * DK], F32, tag="stg")
                    nc.sync.dma_start(
                        wstg[:],
                        wk.rearrange("(co p) n -> p co n", p=P)[
                            :, co4 * 2 : (co4 + 1) * 2, :
                        ],
                    )
                    nc.vector.tensor_copy(
                        wk_sb[:, co4 * 2 : (co4 + 1) * 2, :], wstg[:]
                    )
                wv_sb = wbig_pool.tile([P, CO, DM], BF16, tag="wbig")
                for co4 in range(8):
                    wstg = stg_pool.tile([P, 1, DM], F32, tag="stg")
                    nc.sync.dma_start(
                        wstg[:],
                        wv.rearrange("(co p) n -> p co n", p=P)[:, co4 : co4 + 1, :],
                    )
                    nc.vector.tensor_copy(wv_sb[:, co4 : co4 + 1, :], wstg[:])

                v_sb = v_pool.tile([P, SKV // P, DM], BF16, tag="v")
                state["v_sb"] = v_sb

                # ---- ph1: per 512-row enc block: transpose, K/V proj ----
                for blk in range(SKV // 512):
                    if blk == 0:
                        stg = stg0
                    else:
                        stg = []
                        for t in range(4):
                            sg = stg_pool.tile([P, 1, DM], F32, tag="stg")
                            nc.sync.dma_start(
                                sg[:, 0, :],
                                enc[blk * 512 + t * P : blk * 512 + (t + 1) * P],
                            )
                            stg.append(sg)
                    et = tblk_pool.tile([P, CO, 512], BF16, tag="tblk")
                    sgb = []
                    for t in range(4):
                        sb = exp_pool.tile([P, 2, 512], BF16, tag="exp")
                        nc.vector.tensor_copy(
                            sb[:].rearrange("p a b -> p (a b)"), stg[t][:, 0, :]
                        )
                        sgb.append(sb[:].rearrange("p a b -> p (a b)"))
                    for co in range(CO):
                        tp = ps_aux.tile([P, 4, P], BF16, tag="aux")
                        for t in range(4):
                            nc.tensor.transpose(
                                tp[:, t, :],
                                sgb[t][:, co * P : (co + 1) * P],
                                ident[:],
                            )
                        nc.vector.tensor_copy(et[:, co, :], tp[:])
                    # K^T proj: psum [128 pair-d, 512 kv] -> kt_dram
                    for hp in range(HP):
                        kp = ps_mm.tile([P, 512], F32, tag="mm512")
                        for co in range(CO):
                            nc.tensor.matmul(
                                kp[:],
                                lhsT=wk_sb[:, co, hp * P : (hp + 1) * P],
                                rhs=et[:, co, :],
                                start=(co == 0),
                                stop=(co == CO - 1),
                            )
                        ktw = ktw_pool.tile([P, 512], BF16, tag="ktw")
                        nc.scalar.copy(ktw[:], kp[:])
                        nc.sync.dma_start(
                            kt_dram[hp, :, blk * 512 : (blk + 1) * 512], ktw[:]
                        )
                    # V proj: psum [128 kv, 512 dv]
                    for dvh in range(2):
                        for t in range(4):
                            vp = ps_mm.tile([P, 512], F32, tag="mm512")
                            for co in range(CO):
                                nc.tensor.matmul(
                                    vp[:],
                                    lhsT=et[:, co, t * P : (t + 1) * P],
                                    rhs=wv_sb[:, co, dvh * 512 : (dvh + 1) * 512],
                                    start=(co == 0),
                                    stop=(co == CO - 1),
                                )
                            nc.vector.tensor_copy(
                                v_sb[
                                    :,
                                    blk * 4 + t,
                                    dvh * 512 : (dvh + 1) * 512,
                                ],
                                vp[:],
                            )

            def ph23():
                # ---- ph2/3: transpose pre halves, Q^T proj (bf16) ----
                wq_sb = wk_pool.tile([P, CO, H * DK], BF16, tag="wk")
                for co4 in range(4):
                    wstg = stg_pool.tile([P, 2, H * DK], F32, tag="stg")
                    nc.sync.dma_start(
                        wstg[:],
                        wq.rearrange("(co p) n -> p co n", p=P)[
                            :, co4 * 2 : (co4 + 1) * 2, :
                        ],
                    )
                    nc.vector.tensor_copy(
                        wq_sb[:, co4 * 2 : (co4 + 1) * 2, :], wstg[:]
                    )
                qt_sb = qt_pool.tile([P, HP, SQH], BF16, tag="qt")
                state["qt_sb"] = qt_sb
                for qc in range(2):
                    stg = []
                    for t in range(4):
                        sg = stg_pool.tile([P, 1, DM], F32, tag="stg")
                        nc.sync.dma_start(
                            sg[:, 0, :],
                            pre[qc * 512 + t * P : qc * 512 + (t + 1) * P],
                        )
                        stg.append(sg)
                    pt = tblk_pool.tile([P, CO, 512], BF16, tag="tblk")
                    sgb = []
                    for t in range(4):
                        sb = exp_pool.tile([P, 2, 512], BF16, tag="exp")
                        nc.vector.tensor_copy(
                            sb[:].rearrange("p a b -> p (a b)"), stg[t][:, 0, :]
                        )
                        sgb.append(sb[:].rearrange("p a b -> p (a b)"))
                    for co in range(CO):
                        tp = ps_aux.tile([P, 4, P], BF16, tag="aux")
                        for t in range(4):
                            nc.tensor.transpose(
                                tp[:, t, :],
                                sgb[t][:, co * P : (co + 1) * P],
                                ident[:],
                            )
                        nc.vector.tensor_copy(pt[:, co, :], tp[:])
                    for hp in range(HP):
                        qp = ps_mm.tile([P, 512], F32, tag="mm512")
                        for co in range(CO):
                            nc.tensor.matmul(
                                qp[:],
                                lhsT=wq_sb[:, co, hp * P : (hp + 1) * P],
                                rhs=pt[:, co, :],
                                start=(co == 0),
                                stop=(co == CO - 1),
                            )
                        nc.scalar.copy(
                            qt_sb[:, hp, qc * 512 : (qc + 1) * 512], qp[:]
                        )

            def phwo():
                # ---- Wo load (f32 -> f32r round via DVE) ----
                wo_r = wbig_pool.tile([P, CO, DM], F32R, tag="wbig")
                state["wo_r"] = wo_r
                for co4 in range(8):
                    wstg = stg_pool.tile([P, 1, DM], F32, tag="stg")
                    nc.sync.dma_start(
                        wstg[:],
                        wo.rearrange("(co p) n -> p co n", p=P)[:, co4 : co4 + 1, :],
                    )
                    nc.vector.tensor_copy(wo_r[:, co4 : co4 + 1, :], wstg[:])

            def ph4():
                # ---- ph4: attention ----
                kt_dram, v_sb, qt_sb = state["kt_dram"], state["v_sb"], state["qt_sb"]
                ot_sb = ot_pool.tile([P, CO, SQH], F32R, tag="potr")
                state["ot_sb"] = ot_sb
                SKEW = 2
                pending = []  # (sm, otp, h, qsl, kvh, ex) across heads

                def consume(sm, otp, h, qsl, kvh, ex):
                    for j in range(2):
                        kvc = 2 * kvh + j
                        nc.tensor.matmul(
                            sm[:],
                            lhsT=ones[:],
                            rhs=ex[:, j, :],
                            start=(kvc == 0),
                            stop=(kvc == 15),
                        )
                        nc.tensor.matmul(
                            otp[:],
                            lhsT=v_sb[:, kvc, h * DV : (h + 1) * DV],
                            rhs=ex[:, j, :],
                            start=(kvc == 0),
                            stop=(kvc == 15),
                        )
                    if kvh == 7:
                        # normalization tail for head h
                        rr = r_pool.tile([1, 512], F32, tag="r")
                        nc.vector.reciprocal(rr[:], sm[:])
                        rb = r_pool.tile([P, 512], F32, tag="rb")
                        nc.gpsimd.partition_broadcast(rb[:], rr[:])
                        nc.vector.tensor_mul(ot_sb[:, h, qsl], otp[:], rb[:])

                for qc in range(2):
                    qsl = slice(qc * 512, (qc + 1) * 512)
                    for h in range(H):
                        hp, odd = h // 2, h % 2
                        base = 64 * odd
                        if odd == 0:
                            ktp = ktst_pool.tile([P, SKV], BF16, tag="ktst")
                            nc.sync.dma_start(ktp[:], kt_dram[hp])
                        sm = ps_aux.tile([1, 512], F32, tag="aux")
                        otp = ps_mm.tile([P, 512], F32, tag="mm512")
                        for kvh in range(8):
                            st = ps_st.tile([P, 2, 512], F32, tag="st")
                            for j in range(2):
                                kvc = 2 * kvh + j
                                nc.tensor.matmul(
                                    st[:, j, :],
                                    lhsT=ktp[
                                        base : base + 64,
                                        kvc * P : (kvc + 1) * P,
                                    ],
                                    rhs=qt_sb[base : base + 64, hp, qsl],
                                    start=True,
                                    stop=True,
                                )
                            ex = exp_pool.tile([P, 2, 512], BF16, tag="exp")
                            nc.scalar.activation(
                                ex[:],
                                st[:],
                                mybir.ActivationFunctionType.Exp,
                                bias=0.0,
                                scale=float(EXP_SCALE),
                            )
                            pending.append((sm, otp, h, qsl, kvh, ex))
                            if len(pending) > SKEW:
                                consume(*pending.pop(0))
                for item in pending:
                    consume(*item)
                pending = []

            def ph5():
                # ---- ph5: Y = OT.T @ Wo ----
                ot_sb, wo_r = state["ot_sb"], state["wo_r"]
                for n2 in range(2):
                    nsl = slice(n2 * 512, (n2 + 1) * 512)
                    for qt in range(SQH // P):
                        yp = ps_mm.tile([P, 512], F32, tag="mm512")
                        for hc in range(CO):
                            nc.tensor.matmul(
                                yp[:],
                                lhsT=ot_sb[:, hc, qt * P : (qt + 1) * P],
                                rhs=wo_r[:, hc, nsl],
                                start=(hc == 0),
                                stop=(hc == CO - 1),
                            )
                        ty = y_pool.tile([P, 512], F32, tag="y")
                        nc.scalar.copy(ty[:], yp[:])
                        nc.sync.dma_start(
                            out[qt * P : (qt + 1) * P, nsl], ty[:]
                        )
            phases = [("p1", ph1), ("p23", ph23), ("wo", phwo), ("p4", ph4), ("p5", ph5)]
            if loop_phase == "none":
                for _, f in phases:
                    f()
            elif loop_phase == "all":
                with tc.For_i(0, n_val, 1) as _i:
                    for _, f in phases:
                        f()
            else:
                for name, f in phases:
                    if name == loop_phase:
                        with tc.For_i(0, n_val, 1) as _i:
                            f()
                    else:
                        f()
    nc.finalize()
    return nc


_NC_CACHE = None


def _get_nc():
    global _NC_CACHE
    if _NC_CACHE is None:
        _NC_CACHE = build()
    return _NC_CACHE


def run_sharded(inputs: dict, n_iters: int = 1):
    """Shard full inputs over 8 cores, run, gather full output.

    Returns (full_output [B,SQ,DM] f32, raw BassKernelResults).
    """
    enc_full = np.ascontiguousarray(np.asarray(inputs["encoder_output"], dtype=np.float32))
    pre_full = np.ascontiguousarray(np.asarray(inputs["pre_output"], dtype=np.float32))
    wq = np.ascontiguousarray(np.asarray(inputs["Wq"], dtype=np.float32))
    wk = np.ascontiguousarray(np.asarray(inputs["Wk"], dtype=np.float32))
    wv = np.ascontiguousarray(np.asarray(inputs["Wv"], dtype=np.float32))
    wo = np.ascontiguousarray(np.asarray(inputs["Wo"], dtype=np.float32))
    nit = np.array([[n_iters]], dtype=np.uint32)

    in_maps = []
    for c in range(N_CORES):
        b, qh = c // 2, c % 2
        in_maps.append(
            {
                "enc": enc_full[b],
                "pre": pre_full[b, qh * SQH : (qh + 1) * SQH],
                "wq": wq,
                "wk": wk,
                "wv": wv,
                "wo": wo,
                "n_it": nit,
            }
        )
    res = run_bass_kernel_spmd(_get_nc(), in_maps, list(range(N_CORES)))
    full = np.empty((B, SQ, DM), dtype=np.float32)
    for c in range(N_CORES):
        b, qh = c // 2, c % 2
        full[b, qh * SQH : (qh + 1) * SQH] = res.results[c]["out"]
    return full, res


def kernel(**inputs) -> np.ndarray:
    full, _ = run_sharded(inputs, n_iters=1)
    return full


# revision 12
# speedup vs baseline: 1.0065x; 1.0065x over previous
"""Cross multi-head attention Trainium2 Bass kernel.

Problem: nn_CrossMutiHeadAttention (B=4, SQ=SKV=2048, d_model=1024, H=8,
d_k=64, d_v=128), fp32 in/out.

Sharding (8 cores, no collectives): core c handles batch c//2 and query-row
half c%2 — each core computes K/V projections for its batch (duplicated
across the 2 cores sharing a batch) plus attention + output projection for
its 1024 query rows.

Per-core pipeline (everything SPMD-identical, host shards the inputs):
  ph1: transpose enc (PE) -> encT; project K^T (-> DRAM scratch, bf16) and
       V (-> SBUF, bf16) per 512-row block.
  ph2/3: transpose pre -> preT half; project Q^T (bf16, head-pair packed).
  ph4: per (q-half, head): S^T = K^T.T@Q^T into PSUM (f32), exp via ACT
       (scale=1/8) -> bf16, denominator via ones-matmul, O^T = V.T@P^T
       accumulated over kv chunks, normalize with broadcast reciprocal
       (-> f32r).
  ph5: Y = O^T.T @ Wo (f32r), ACT copy -> f32, DMA out.

The whole body sits in a For_i whose trip count is a runtime input so
test.py can measure steady-state device time via wall-clock deltas.
"""

from contextlib import ExitStack

import numpy as np

import concourse.bass as bass
import concourse.mybir as mybir
from concourse import bacc
from concourse.bass_utils import run_bass_kernel_spmd
from concourse.masks import make_identity
from concourse.tile import TileContext

F32 = mybir.dt.float32
F32R = mybir.dt.float32r
BF16 = mybir.dt.bfloat16

P = 128
B, SQ, SKV, DM = 4, 2048, 2048, 1024
H, DK, DV = 8, 64, 128
SQH = SQ // 2          # 1024 query rows per core
HP = H // 2            # 4 head pairs
CO = DM // P           # 8 contraction chunks
N_CORES = 8

EXP_SCALE = 1.0 / np.sqrt(DK).astype(np.float32)  # 0.125


def build(loop_phase="all"):
    nc = bacc.Bacc()
    enc = nc.declare_dram_parameter("enc", [SKV, DM], F32, isOutput=False)
    pre = nc.declare_dram_parameter("pre", [SQH, DM], F32, isOutput=False)
    wq = nc.declare_dram_parameter("wq", [DM, H * DK], F32, isOutput=False)
    wk = nc.declare_dram_parameter("wk", [DM, H * DK], F32, isOutput=False)
    wv = nc.declare_dram_parameter("wv", [DM, DM], F32, isOutput=False)
    wo = nc.declare_dram_parameter("wo", [DM, DM], F32, isOutput=False)
    n_it = nc.declare_dram_parameter("n_it", [1, 1], mybir.dt.uint32, isOutput=False)
    out = nc.declare_dram_parameter("out", [SQH, DM], F32, isOutput=True)

    with ExitStack() as ctx:
        tc = ctx.enter_context(TileContext(nc))
        ec = ctx.enter_context
        if True:
            cpool = ec(tc.tile_pool(name="const", bufs=1))
            stg_pool = ec(tc.tile_pool(name="stg", bufs=8))
            wk_pool = ec(tc.tile_pool(name="wk", bufs=1))
            wbig_pool = ec(tc.tile_pool(name="wbig", bufs=1))
            ktst_pool = ec(tc.tile_pool(name="ktst", bufs=1))
            ktw_pool = ec(tc.tile_pool(name="ktw", bufs=2))
            qt_pool = ec(tc.tile_pool(name="qt", bufs=1))
            tblk_pool = ec(tc.tile_pool(name="tblk", bufs=2))
            v_pool = ec(tc.tile_pool(name="vpool", bufs=1))
            exp_pool = ec(tc.tile_pool(name="exp", bufs=8))
            ot_pool = ec(tc.tile_pool(name="ot", bufs=1))
            y_pool = ec(tc.tile_pool(name="ysb", bufs=2))
            r_pool = ec(tc.tile_pool(name="rsm", bufs=2))
            dram_pool = ec(tc.tile_pool(name="dram", bufs=1, space="DRAM"))
            ps_st = ec(tc.tile_pool(name="ps_st", bufs=2, space="PSUM"))
            ps_mm = ec(tc.tile_pool(name="ps_mm", bufs=2, space="PSUM"))
            ps_aux = ec(tc.tile_pool(name="ps_aux", bufs=2, space="PSUM"))
            ident = cpool.tile([P, P], BF16, tag="ident")
            make_identity(nc, ident[:])
            ones = cpool.tile([P, 1], BF16, tag="ones")
            nc.gpsimd.memset(ones[:], 1.0)
            nit_sb = cpool.tile([1, 1], mybir.dt.uint32, tag="nit")
            nc.sync.dma_start(nit_sb[:], n_it[:])

            regs = []
            for eng_t in mybir.ALL_ENGINES:
                r = nc.alloc_register(eng_t, f"nit_{eng_t.name}")
                nc.engines[eng_t].reg_load(r, nit_sb[0:1, 0:1])
                regs.append(r)
            n_val = bass.RegisterHandles(regs)

            state = {}

            def ph1():
                # K^T resident: [128 (d of 2 heads), pair, SKV] bf16
                kt_sb = ktst_pool.tile([P, HP, SKV], BF16, tag="ktst")
                state["kt_sb"] = kt_sb
                # first enc block DMA goes out ahead of the weight loads so
                # PE transposes can start immediately
                stg0 = []
                for t in range(4):
                    sg = stg_pool.tile([P, 1, DM], F32, tag="stg")
                    nc.sync.dma_start(
                        sg[:, 0, :],
                        enc[t * P : (t + 1) * P],
                    )
                    stg0.append(sg)
                # ---- weights for ph1 (f32 DMA + DVE cast to bf16) ----
                wk_sb = wk_pool.tile([P, CO, H * DK], BF16, tag="wk")
                for co4 in range(4):
                    wstg = stg_pool.tile([P, 2, H * DK], F32, tag="stg")
                    nc.sync.dma_start(
                        wstg[:],
                        wk.rearrange("(co p) n -> p co n", p=P)[
                            :, co4 * 2 : (co4 + 1) * 2, :
                        ],
                    )
                    nc.vector.tensor_copy(
                        wk_sb[:, co4 * 2 : (co4 + 1) * 2, :], wstg[:]
                    )
                wv_sb = wbig_pool.tile([P, CO, DM], BF16, tag="wbig")
                for co4 in range(8):
                    wstg = stg_pool.tile([P, 1, DM], F32, tag="stg")
                    nc.sync.dma_start(
                        wstg[:],
                        wv.rearrange("(co p) n -> p co n", p=P)[:, co4 : co4 + 1, :],
                    )
                    nc.vector.tensor_copy(wv_sb[:, co4 : co4 + 1, :], wstg[:])

                v_sb = v_pool.tile([P, SKV // P, DM], BF16, tag="v")
                state["v_sb"] = v_sb

                # ---- ph1: per 512-row enc block: transpose, K/V proj ----
                for blk in range(SKV // 512):
                    if blk == 0:
                        stg = stg0
                    else:
                        stg = []
                        for t in range(4):
                            sg = stg_pool.tile([P, 1, DM], F32, tag="stg")
                            nc.sync.dma_start(
                                sg[:, 0, :],
                                enc[blk * 512 + t * P : blk * 512 + (t + 1) * P],
                            )
                            stg.append(sg)
                    et = tblk_pool.tile([P, CO, 512], BF16, tag="tblk")
                    sgb = []
                    for t in range(4):
                        sb = exp_pool.tile([P, 2, 512], BF16, tag="exp")
                        nc.vector.tensor_copy(
                            sb[:].rearrange("p a b -> p (a b)"), stg[t][:, 0, :]
                        )
                        sgb.append(sb[:].rearrange("p a b -> p (a b)"))
                    for co in range(CO):
                        tp = ps_aux.tile([P, 4, P], BF16, tag="aux")
                        for t in range(4):
                            nc.tensor.transpose(
                                tp[:, t, :],
                                sgb[t][:, co * P : (co + 1) * P],
                                ident[:],
                            )
                        nc.vector.tensor_copy(et[:, co, :], tp[:])
                    # K^T proj: psum [128 pair-d, 512 kv] -> kt_dram
                    for hp in range(HP):
                        kp = ps_mm.tile([P, 512], F32, tag="mm512")
                        for co in range(CO):
                            nc.tensor.matmul(
                                kp[:],
                                lhsT=wk_sb[:, co, hp * P : (hp + 1) * P],
                                rhs=et[:, co, :],
                                start=(co == 0),
                                stop=(co == CO - 1),
                            )
                        nc.scalar.copy(
                            kt_sb[:, hp, blk * 512 : (blk + 1) * 512], kp[:]
                        )
                    # V proj: psum [128 kv, 512 dv]
                    for dvh in range(2):
                        for t in range(4):
                            vp = ps_mm.tile([P, 512], F32, tag="mm512")
                            for co in range(CO):
                                nc.tensor.matmul(
                                    vp[:],
                                    lhsT=et[:, co, t * P : (t + 1) * P],
                                    rhs=wv_sb[:, co, dvh * 512 : (dvh + 1) * 512],
                                    start=(co == 0),
                                    stop=(co == CO - 1),
                                )
                            nc.vector.tensor_copy(
                                v_sb[
                                    :,
                                    blk * 4 + t,
                                    dvh * 512 : (dvh + 1) * 512,
                                ],
                                vp[:],
                            )

            def ph23():
                # ---- ph2/3: transpose pre halves, Q^T proj (bf16) ----
                wq_sb = wk_pool.tile([P, CO, H * DK], BF16, tag="wk")
                for co4 in range(4):
                    wstg = stg_pool.tile([P, 2, H * DK], F32, tag="stg")
                    nc.sync.dma_start(
                        wstg[:],
                        wq.rearrange("(co p) n -> p co n", p=P)[
                            :, co4 * 2 : (co4 + 1) * 2, :
                        ],
                    )
                    nc.vector.tensor_copy(
                        wq_sb[:, co4 * 2 : (co4 + 1) * 2, :], wstg[:]
                    )
                qt_sb = qt_pool.tile([P, HP, SQH], BF16, tag="qt")
                state["qt_sb"] = qt_sb
                for qc in range(2):
                    stg = []
                    for t in range(4):
                        sg = stg_pool.tile([P, 1, DM], F32, tag="stg")
                        nc.sync.dma_start(
                            sg[:, 0, :],
                            pre[qc * 512 + t * P : qc * 512 + (t + 1) * P],
                        )
                        stg.append(sg)
                    pt = tblk_pool.tile([P, CO, 512], BF16, tag="tblk")
                    sgb = []
                    for t in range(4):
                        sb = exp_pool.tile([P, 2, 512], BF16, tag="exp")
                        nc.vector.tensor_copy(
                            sb[:].rearrange("p a b -> p (a b)"), stg[t][:, 0, :]
                        )
                        sgb.append(sb[:].rearrange("p a b -> p (a b)"))
                    for co in range(CO):
                        tp = ps_aux.tile([P, 4, P], BF16, tag="aux")
                        for t in range(4):
                            nc.tensor.transpose(
                                tp[:, t, :],
                                sgb[t][:, co * P : (co + 1) * P],
                                ident[:],
                            )
                        nc.vector.tensor_copy(pt[:, co, :], tp[:])
                    for hp in range(HP):
                        qp = ps_mm.tile([P, 512], F32, tag="mm512")
                        for co in range(CO):
                            nc.tensor.matmul(
                                qp[:],
                                lhsT=wq_sb[:, co, hp * P : (hp + 1) * P],
                                rhs=pt[:, co, :],
                                start=(co == 0),
                                stop=(co == CO - 1),
                            )
                        nc.scalar.copy(
                            qt_sb[:, hp, qc * 512 : (qc + 1) * 512], qp[:]
                        )

            def phwo():
                # ---- Wo load (f32 -> f32r round via DVE) ----
                wo_r = wbig_pool.tile([P, CO, DM], F32R, tag="wbig")
                state["wo_r"] = wo_r
                for co4 in range(8):
                    wstg = stg_pool.tile([P, 1, DM], F32, tag="stg")
                    nc.sync.dma_start(
                        wstg[:],
                        wo.rearrange("(co p) n -> p co n", p=P)[:, co4 : co4 + 1, :],
                    )
                    nc.vector.tensor_copy(wo_r[:, co4 : co4 + 1, :], wstg[:])

            def ph4():
                # ---- ph4: attention ----
                kt_sb, v_sb, qt_sb = state["kt_sb"], state["v_sb"], state["qt_sb"]
                ot_sb = ot_pool.tile([P, CO, SQH], F32R, tag="potr")
                state["ot_sb"] = ot_sb
                SKEW = 2
                pending = []  # (sm, otp, h, qsl, kvh, ex) across heads

                def consume(sm, otp, h, qsl, kvh, ex):
                    for j in range(2):
                        kvc = 2 * kvh + j
                        nc.tensor.matmul(
                            sm[:],
                            lhsT=ones[:],
                            rhs=ex[:, j, :],
                            start=(kvc == 0),
                            stop=(kvc == 15),
                        )
                        nc.tensor.matmul(
                            otp[:],
                            lhsT=v_sb[:, kvc, h * DV : (h + 1) * DV],
                            rhs=ex[:, j, :],
                            start=(kvc == 0),
                            stop=(kvc == 15),
                        )
                    if kvh == 7:
                        # normalization tail for head h
                        rr = r_pool.tile([1, 512], F32, tag="r")
                        nc.vector.reciprocal(rr[:], sm[:])
                        rb = r_pool.tile([P, 512], F32, tag="rb")
                        nc.gpsimd.partition_broadcast(rb[:], rr[:])
                        nc.vector.tensor_mul(ot_sb[:, h, qsl], otp[:], rb[:])

                for qc in range(2):
                    qsl = slice(qc * 512, (qc + 1) * 512)
                    for h in range(H):
                        hp, odd = h // 2, h % 2
                        base = 64 * odd
                        ktp = kt_sb[:, hp, :]
                        sm = ps_aux.tile([1, 512], F32, tag="aux")
                        otp = ps_mm.tile([P, 512], F32, tag="mm512")
                        for kvh in range(8):
                            st = ps_st.tile([P, 2, 512], F32, tag="st")
                            for j in range(2):
                                kvc = 2 * kvh + j
                                nc.tensor.matmul(
                                    st[:, j, :],
                                    lhsT=ktp[
                                        base : base + 64,
                                        kvc * P : (kvc + 1) * P,
                                    ],
                                    rhs=qt_sb[base : base + 64, hp, qsl],
                                    start=True,
                                    stop=True,
                                )
                            ex = exp_pool.tile([P, 2, 512], BF16, tag="exp")
                            nc.scalar.activation(
                                ex[:],
                                st[:],
                                mybir.ActivationFunctionType.Exp,
                                bias=0.0,
                                scale=float(EXP_SCALE),
                            )
                            pending.append((sm, otp, h, qsl, kvh, ex))
                            if len(pending) > SKEW:
                                consume(*pending.pop(0))
                for item in pending:
                    consume(*item)
                pending = []

            def ph5():
                # ---- ph5: Y = OT.T @ Wo ----
                ot_sb, wo_r = state["ot_sb"], state["wo_r"]
                for n2 in range(2):
                    nsl = slice(n2 * 512, (n2 + 1) * 512)
                    for qt in range(SQH // P):
                        yp = ps_mm.tile([P, 512], F32, tag="mm512")
                        for hc in range(CO):
                            nc.tensor.matmul(
                                yp[:],
                                lhsT=ot_sb[:, hc, qt * P : (qt + 1) * P],
                                rhs=wo_r[:, hc, nsl],
                                start=(hc == 0),
                                stop=(hc == CO - 1),
                            )
                        ty = y_pool.tile([P, 512], F32, tag="y")
                        nc.scalar.copy(ty[:], yp[:])
                        nc.sync.dma_start(
                            out[qt * P : (qt + 1) * P, nsl], ty[:]
                        )
            phases = [("p1", ph1), ("p23", ph23), ("wo", phwo), ("p4", ph4), ("p5", ph5)]
            if loop_phase == "none":
                for _, f in phases:
                    f()
            elif loop_phase == "all":
                with tc.For_i(0, n_val, 1) as _i:
                    for _, f in phases:
                        f()
            else:
                for name, f in phases:
                    if name == loop_phase:
                        with tc.For_i(0, n_val, 1) as _i:
                            f()
                    else:
                        f()
    nc.finalize()
    return nc


_NC_CACHE = None


def _get_nc():
    global _NC_CACHE
    if _NC_CACHE is None:
        _NC_CACHE = build()
    return _NC_CACHE


def run_sharded(inputs: dict, n_iters: int = 1):
    """Shard full inputs over 8 cores, run, gather full output.

    Returns (full_output [B,SQ,DM] f32, raw BassKernelResults).
    """
    enc_full = np.ascontiguousarray(np.asarray(inputs["encoder_output"], dtype=np.float32))
    pre_full = np.ascontiguousarray(np.asarray(inputs["pre_output"], dtype=np.float32))
    wq = np.ascontiguousarray(np.asarray(inputs["Wq"], dtype=np.float32))
    wk = np.ascontiguousarray(np.asarray(inputs["Wk"], dtype=np.float32))
    wv = np.ascontiguousarray(np.asarray(inputs["Wv"], dtype=np.float32))
    wo = np.ascontiguousarray(np.asarray(inputs["Wo"], dtype=np.float32))
    nit = np.array([[n_iters]], dtype=np.uint32)

    in_maps = []
    for c in range(N_CORES):
        b, qh = c // 2, c % 2
        in_maps.append(
            {
                "enc": enc_full[b],
                "pre": pre_full[b, qh * SQH : (qh + 1) * SQH],
                "wq": wq,
                "wk": wk,
                "wv": wv,
                "wo": wo,
                "n_it": nit,
            }
        )
    res = run_bass_kernel_spmd(_get_nc(), in_maps, list(range(N_CORES)))
    full = np.empty((B, SQ, DM), dtype=np.float32)
    for c in range(N_CORES):
        b, qh = c // 2, c % 2
        full[b, qh * SQH : (qh + 1) * SQH] = res.results[c]["out"]
    return full, res


def kernel(**inputs) -> np.ndarray:
    full, _ = run_sharded(inputs, n_iters=1)
    return full
